# revision 1
# baseline (speedup 1.0000x reference)
"""Trainium2 Bass kernel for the DiffeqSolver problem.

Math: the reference solves dy/dt = tanh(y@W1+b1)@W2+b2 (autonomous) with
adaptive dopri5 at rtol=1e-4 for 24 per-batch time points. A single RK2
(explicit midpoint) step per output interval reproduces the reference to
~3.6e-4 relative -- two orders of magnitude inside the 2e-2 gate -- at half
the engine work of RK4, so the kernel runs 23 midpoint steps.

The midpoint stage is fused through matmul associativity: with
q1 = y@W1, a1 = tanh(q1+b1), the stage-2 pre-activation is
  q2 = (y + h/2*(a1@W2+b2))@W1 + b1
     = q1 + (h/2)*a1@(W2@W1) + [b1 + (h/2)*b2@W1]
so the kernel never materializes the midpoint state: it accumulates
(h/2)*a1@V (V = W2@W1, host-scaled per batch/interval -- weight loads are
free) directly onto the stage-1 PSUM tile and applies the bias inside the
stage-2 tanh.  Per interval and pair that leaves 6 matmuls, 2 tanhs and a
single DVE state update y' = y + h*(a2@W2).

Distribution: data-parallel over the batch axis -- 8 batches per NeuronCore
in 4 pairs.  The pair state lives in one SBUF tile [128, 326] f32r
(partitions 0:64 = batch A latent dims transposed, 64:128 = batch B; free
dim padded to 326 because f32r matmuls need an even moving dim).  mm1/mm2
use block-extended [128,128] weights so every matmul writes PSUM at
partition 0; the V matmuls contract the full 128 hidden dims.  Each pair
owns two private banks of one 8-bank PSUM tile for the whole run: q1/q2
accumulate in place and k2 reuses the bank after the stage-2 tanh has read
it, so the four pair-chains share nothing but the engines.  State
writeback is one DMA per interval and pair (3-level dram access pattern
interleaving the two batch halves), split across the SP and Pool DMA
queues by pair parity.

Scheduling: the tile scheduler's ordering pass runs on a simulation whose
timing diverges from the grading cost model, so left alone it emits
depth-first per-chain engine orders that head-of-line block every in-order
engine stream (observed ~50% ACT idle).  The kernel instead pins every
PE/ACT/DVE stream order with no-sync (order-only) dependency edges,
software-pipelining the four chains so the bottleneck ACT engine (tanh is
ACT-only; ~130 us total) runs back-to-back: ACT ladder
[a1_01-merged, a1_2, a2_0, a2_1, a1_3, a2_2, a2_3] per interval, with
next-interval q1 matmuls and pair 3's K/update deferred into the slots
where their inputs are ready.  Pairs 0/1's stage-1 tanh is one merged ACT
op over a uniform-stride 4-region PSUM view to amortize the ~185 ns
per-op access latency.  The measured schedule holds ACT >99.9% busy
between the first and last tanh.
"""

import numpy as np
from contextlib import ExitStack

B, P, D, H, T = 64, 325, 64, 128, 24
NCORE = 8
BPC = B // NCORE  # 8 batches per core
NPAIR = BPC // 2  # 4
R = BPC * P  # 2600 rows per core
PF = P + 1  # free-dim padded to even (f32r matmul requires an even moving dim)
RPAD = BPC * PF  # per-core padded y0 width
NV = 4  # coefficient vectors per (pair, interval)
NI = T - 1  # 23 integration intervals

_CACHE = {}


def _coef_tables(ts, W1, b1, b2):
    """Per-core coefficient table [NCORE, 128, NI*NPAIR*NV] fp32.

    Per (interval j, pair p) the NV columns are:
      0: h      (pair-stacked per-partition)   (final state update)
      1: biasA = b1 + (hA/2)*(b2@W1)           (stage-2 tanh bias, batch A)
      2: biasB                                  (same, batch B)
      3: h*b2   (pair-stacked)                  (final combine b2 term)
    Columns 1-3 matter only when b2 != 0 (generic path).
    """
    f32 = np.float32
    dt = np.diff(ts.astype(f32), axis=0)  # [NI, B]
    bw = (b2.astype(f32) @ W1.astype(f32)).astype(f32)  # [H]
    b1f = b1.astype(f32)
    b2f = b2.astype(f32)
    coef = np.zeros((NCORE, 128, NI * NPAIR * NV), f32)
    for c in range(NCORE):
        for j in range(NI):
            for p in range(NPAIR):
                bA = c * BPC + 2 * p
                bB = bA + 1
                hA = dt[j, bA]
                hB = dt[j, bB]
                base = (j * NPAIR + p) * NV
                v = np.zeros((128, NV), f32)
                v[:64, 0] = hA
                v[64:, 0] = hB
                v[:, 1] = b1f + f32(0.5) * hA * bw
                v[:, 2] = b1f + f32(0.5) * hB * bw
                v[:64, 3] = hA * b2f
                v[64:, 3] = hB * b2f
                coef[c, :, base : base + NV] = v
    return coef


def _vsc_tables(ts, W1, W2):
    """Per-core h-scaled V = W2@W1 weight tables
    [NCORE, 128, NI*NPAIR*2*H] fp32: per (interval, pair) the two 128-col
    blocks are (hA/2)*V and (hB/2)*V."""
    f32 = np.float32
    dt = np.diff(ts.astype(f32), axis=0)  # [NI, B]
    V = (W2.astype(f32) @ W1.astype(f32)).astype(f32)  # [H, H]
    vsc = np.zeros((NCORE, 128, NI * NPAIR * 2 * H), f32)
    for c in range(NCORE):
        for j in range(NI):
            for p in range(NPAIR):
                bA = c * BPC + 2 * p
                base = ((j * NPAIR + p) * 2) * H
                vsc[c, :, base : base + H] = f32(0.5) * dt[j, bA] * V
                vsc[c, :, base + H : base + 2 * H] = f32(0.5) * dt[j, bA + 1] * V
    return vsc


def _build_program(fast=False):
    """fast=True is valid when b2 == 0: the stage-2 tanh bias collapses to
    b1 (one two-region ACT op per stage) and the final combine is a single
    scalar_tensor_tensor."""
    key = ("nc", fast)
    if key in _CACHE:
        return _CACHE[key]

    import concourse.bacc as bacc
    import concourse.tile as tile
    import concourse.mybir as mybir

    f32 = mybir.dt.float32
    f32r = mybir.dt.float32r
    AF = mybir.ActivationFunctionType
    OP = mybir.AluOpType

    nc = bacc.Bacc(
        "TRN2",
        target_bir_lowering=False,
        debug=False,
        enable_asserts=False,
        num_devices=NCORE,
    )
    y0_d = nc.dram_tensor("y0", [D, RPAD], f32r, kind="ExternalInput").ap()
    coef_d = nc.dram_tensor("coef", [128, NI * NPAIR * NV], f32, kind="ExternalInput").ap()
    w1ab_d = nc.dram_tensor("w1ab", [128, 2 * H], f32r, kind="ExternalInput").ap()
    w2ab_d = nc.dram_tensor("w2ab", [H, 256], f32r, kind="ExternalInput").ap()
    b1_d = nc.dram_tensor("b1", [H, 1], f32, kind="ExternalInput").ap()
    vsc_d = nc.dram_tensor("vsc", [128, NI * NPAIR * 2 * H], f32r, kind="ExternalInput").ap()
    out_d = nc.dram_tensor("out", [T, D, R], f32, kind="ExternalOutput").ap()

    def out_ap(j, p):
        # [2, 64, 325] view of out[j]: batch-half h, latent dim d, point q
        return out_d[j, :, 2 * p * P : (2 * p + 2) * P].rearrange(
            "d (h q) -> h d q", h=2
        )

    with tile.TileContext(nc) as tc:
        with ExitStack() as ctx:
            const = ctx.enter_context(tc.tile_pool(name="const", bufs=1))
            ypool = ctx.enter_context(tc.tile_pool(name="ypool", bufs=6))
            apool = ctx.enter_context(tc.tile_pool(name="apool", bufs=2))
            tpool = ctx.enter_context(tc.tile_pool(name="tpool", bufs=2))
            gpool = ctx.enter_context(tc.tile_pool(name="gpool", bufs=1, space="PSUM"))

            # Startup DMA order follows the first dependency chain; the W1
            # loads use the (otherwise idle at startup) ACT hwdge queue so
            # they overlap the SP-queue loads.
            w1ab_t = const.tile([128, 2 * H], f32r, name="w1abt")
            nc.scalar.dma_start(out=w1ab_t[:], in_=w1ab_d[:])
            w1a_t = w1ab_t[:, 0:H]
            w1b_t = w1ab_t[:, H : 2 * H]

            # y0 loads split across the SP and Pool queues: the merged
            # stage-1 tanh needs pairs 0 AND 1 loaded, so those two go first
            # on separate queues while the weights load on ACT's queue.
            ytiles = []
            for p in range(NPAIR):
                ytr = ypool.tile([128, PF], f32r, name=f"y{p}", tag=f"y{p}")
                deng = nc.sync if p in (0, 2) else nc.gpsimd
                deng.dma_start(
                    out=ytr[:],
                    in_=y0_d[:, 2 * p * PF : (2 * p + 2) * PF].rearrange(
                        "d (h q) -> h d q", h=2
                    ),
                )
                ytiles.append(ytr)

            # vsc is chunked into separate tiles so interval j's V matmuls
            # only wait for the chunk that covers j.  Chunks 0/1 load at
            # startup; the rest are spread through the interval loop (one
            # chunk every three intervals, two intervals ahead of first
            # use) so their payloads don't monopolize the serialized DMA
            # engines ahead of the state-writeback DMAs.
            vsc_chunks = [1, 2, 3, 3, 3, 3, 3, 3, 2]  # intervals per chunk
            vsc_tiles = []  # (tile, first interval-index covered)
            jc = 0
            for ci, nint in enumerate(vsc_chunks):
                cols = nint * NPAIR * 2 * H
                vt = const.tile([128, cols], f32r, name=f"vsct{ci}")
                vsc_tiles.append((vt, jc, jc * NPAIR * 2 * H, cols))
                jc += nint

            def load_vsc(ci):
                vt, _, c0, cols = vsc_tiles[ci]
                nc.sync.dma_start(out=vt[:], in_=vsc_d[:, c0 : c0 + cols])

            def vsc_slice(j, p):
                # [128, H] blocks for (interval j, pair p), A then B
                ji = j - 1
                for vt, j0, _, _ in reversed(vsc_tiles):
                    if ji >= j0:
                        off = ((ji - j0) * NPAIR + p) * 2 * H
                        return vt[:, off : off + H], vt[:, off + H : off + 2 * H]
                raise AssertionError(j)

            b1_t = const.tile([H, 1], f32, name="b1t")
            nc.sync.dma_start(out=b1_t[:], in_=b1_d[:])
            load_vsc(0)
            coef_t = const.tile([128, NI * NPAIR * NV], f32, name="coeft")
            nc.sync.dma_start(out=coef_t[:], in_=coef_d[:])
            w2ab_t = const.tile([H, 256], f32r, name="w2abt")
            nc.sync.dma_start(out=w2ab_t[:], in_=w2ab_d[:])
            w2a_t = w2ab_t[:, 0:128]
            w2b_t = w2ab_t[:, 128:256]
            load_vsc(1)

            # chunk ci first needed at interval vsc_tiles[ci][1]+1; load it
            # two intervals earlier, from the interval emission loop
            vsc_load_at = {max(1, vsc_tiles[ci][1] - 2): ci for ci in range(2, len(vsc_chunks))}

            cur = list(ytiles)
            # one 8-bank PSUM tile; pair p's two tanh-input regions live at
            # 1024p (batch A) and 1024p+512 (batch B), so pairs 0/1's four
            # regions form a uniform-stride view for a merged stage-1 tanh
            gall = gpool.tile([128, 4096], f32, name="gall", tag="gall")

            # The tile scheduler orders each engine's in-order stream from a
            # scheduling-pass simulation whose timing diverges from the real
            # cost model; left to itself it picks a depth-first order that
            # head-of-line blocks every engine on its own chain (observed
            # ~50% ACT idle).  Force the breadth-first (interval, stage,
            # pair) order with explicit no-sync (order-only) edges instead.
            prev_on = {}

            def seq(engkey, binst):
                ins = binst.ins if hasattr(binst, "ins") else binst
                if engkey in prev_on:
                    ins.add_dependency(
                        prev_on[engkey], mybir.DependencyInfo.NO_SYNC_ONLY
                    )
                prev_on[engkey] = ins.name
                return binst

            # Engine-stream orders are hand-scheduled (via seq) into a
            # software pipeline that keeps ACT -- the bottleneck engine at
            # ~728 ns/op, 8 ops/interval -- 100% busy in steady state.  The
            # ACT "ladder" order a1_0,a1_1,a2_0,a1_2,a2_1,a1_3,a2_2,a2_3
            # starts each pair's loop-carried tail (K -> state update ->
            # next q1) three ACT slots before the interval ends, so the
            # first tanh of interval j+1 is ready the moment ACT drains
            # interval j.  Next-interval q1 matmuls are interleaved into the
            # PE stream right after the K they depend on (via the state
            # update) to keep PE's in-order stream from blocking them behind
            # late Ks.
            def regions(p):
                return gall, gall[:, 1024 * p : 1024 * p + PF], gall[
                    :, 1024 * p + 512 : 1024 * p + 512 + PF
                ]

            def emit_q1(p, y):
                g, gA, gB = regions(p)
                seq('PE', nc.tensor.matmul(gA, w1a_t, y[:], start=True, stop=True))
                seq('PE', nc.tensor.matmul(gB, w1b_t, y[:], start=True, stop=True))

            def emit_a1(p, j):
                a1 = apool.tile([128, 2 * PF], f32r, name=f"a1_{p}", tag=f"a1{p}")
                gview = gall[:, 1024 * p : 1024 * (p + 1)].rearrange(
                    "q (r c) -> q r c", r=2
                )[:, :, 0:PF]
                a1view = a1[:].rearrange("q (r c) -> q r c", r=2)
                seq('ACT', nc.scalar.activation(
                    a1view, gview, AF.Tanh, bias=b1_t[:, 0:1], scale=1.0
                ))
                return a1

            def emit_a1_01(j):
                # merged stage-1 tanh for pairs 0 and 1: one ACT op over the
                # four 512-strided regions in PSUM banks 0-3
                a1 = apool.tile([128, 4 * PF], f32r, name="a1_01", tag="a1m")
                gview = gall[:, 0:2048].rearrange("q (r c) -> q r c", r=4)[:, :, 0:PF]
                a1view = a1[:].rearrange("q (r c) -> q r c", r=4)
                seq('ACT', nc.scalar.activation(
                    a1view, gview, AF.Tanh, bias=b1_t[:, 0:1], scale=1.0
                ))
                return a1

            def emit_v(p, j, aA, aB):
                g, gA, gB = regions(p)
                vA, vB = vsc_slice(j, p)
                seq('PE', nc.tensor.matmul(
                    gA, vA, aA,
                    start=False, stop=True, skip_group_check=True,
                ))
                seq('PE', nc.tensor.matmul(
                    gB, vB, aB,
                    start=False, stop=True, skip_group_check=True,
                ))

            def emit_a2(p, j):
                g, gA, gB = regions(p)
                base = ((j - 1) * NPAIR + p) * NV
                a2 = apool.tile([128, 2 * PF], f32r, name=f"a2_{p}", tag=f"a2{p}")
                if fast:
                    gview = gall[:, 1024 * p : 1024 * (p + 1)].rearrange(
                        "q (r c) -> q r c", r=2
                    )[:, :, 0:PF]
                    a2view = a2[:].rearrange("q (r c) -> q r c", r=2)
                    seq('ACT', nc.scalar.activation(
                        a2view, gview, AF.Tanh, bias=b1_t[:, 0:1], scale=1.0
                    ))
                else:
                    seq('ACT', nc.scalar.activation(
                        a2[:, 0:PF], gA, AF.Tanh,
                        bias=coef_t[:, base + 1 : base + 2], scale=1.0,
                    ))
                    seq('ACT', nc.scalar.activation(
                        a2[:, PF : 2 * PF], gB, AF.Tanh,
                        bias=coef_t[:, base + 2 : base + 3], scale=1.0,
                    ))
                return a2

            def emit_k(p, a2):
                g, gA, gB = regions(p)
                kv = gA
                seq('PE', nc.tensor.matmul(kv, w2a_t, a2[:, 0:PF], start=True, stop=False))
                seq('PE', nc.tensor.matmul(
                    kv, w2b_t, a2[:, PF : 2 * PF], start=False, stop=True
                ))

            def emit_upd(p, j, y):
                # state update + writeback; k2 is read from the pair's bank
                g, gA, gB = regions(p)
                kv = gA
                base = ((j - 1) * NPAIR + p) * NV
                hvec = coef_t[:, base : base + 1]
                ynew = ypool.tile([128, PF], f32r, name=f"y{p}", tag=f"y{p}")
                if fast:
                    seq('DVE', nc.vector.scalar_tensor_tensor(
                        ynew[:], kv, hvec, y[:].bitcast(f32), OP.mult, OP.add
                    ))
                else:
                    tmp = tpool.tile([128, PF], f32, name=f"t{p}", tag=f"t{p}")
                    seq('DVE', nc.vector.tensor_scalar(
                        tmp[:], kv, hvec, coef_t[:, base + 3 : base + 4],
                        OP.mult, OP.add,
                    ))
                    seq('DVE', nc.vector.tensor_tensor(
                        ynew[:], tmp[:], y[:].bitcast(f32), OP.add
                    ))
                cur[p] = ynew
                # split output DMAs across two queues (SP hwdge / Pool swdge)
                # so one pair's late state doesn't head-of-line block the
                # other pairs' writebacks; the final interval's writebacks
                # all take the faster hwdge path (SP is drained by then) to
                # shorten the kernel tail
                deng = nc.sync if (p < 2 or j == T - 1) else nc.gpsimd
                deng.dma_start(out=out_ap(j, p), in_=ynew[:, 0:P].bitcast(f32))

            # The PE stream is ordered by steady-state ready time, with each
            # op ahead of its deadline: pairs 0/1's next-interval q1 runs
            # inside the current interval (right after the K/stt that feeds
            # it); pairs 2/3's q1 -- and pair 3's K/state-update, whose a2
            # only finishes at the interval boundary -- carry over as
            # leftovers at the top of the next interval.
            # prologue: q1 for interval 1
            for p in range(NPAIR):
                emit_q1(p, cur[p])
            pend3 = None  # (j, a2_3, y3) whose K/update is deferred
            for j in range(1, T):
                ys = list(cur)
                last = j == T - 1
                if j == 2:
                    # the t=0 outputs, deferred past startup so their
                    # payloads don't contend with the first vsc/coef loads
                    for p in range(NPAIR):
                        nc.sync.dma_start(
                            out=out_ap(0, p), in_=ytiles[p][:, 0:P].bitcast(f32)
                        )
                if j in vsc_load_at:
                    load_vsc(vsc_load_at[j])
                if pend3 is not None:
                    pj, pa2, py = pend3
                    emit_k(3, pa2)
                    emit_upd(3, pj, py)
                    ys[3] = cur[3]
                if j >= 2:
                    emit_q1(2, ys[2])
                a1m = emit_a1_01(j)
                emit_v(0, j, a1m[:, 0:PF], a1m[:, PF : 2 * PF])
                emit_v(1, j, a1m[:, 2 * PF : 3 * PF], a1m[:, 3 * PF : 4 * PF])
                if j >= 2:
                    emit_q1(3, ys[3])
                a1_2 = emit_a1(2, j)
                emit_v(2, j, a1_2[:, 0:PF], a1_2[:, PF : 2 * PF])
                a2_0 = emit_a2(0, j)
                emit_k(0, a2_0)
                a2_1 = emit_a2(1, j)
                emit_k(1, a2_1)
                emit_upd(0, j, ys[0])
                if not last:
                    emit_q1(0, cur[0])
                a1_3 = emit_a1(3, j)
                emit_v(3, j, a1_3[:, 0:PF], a1_3[:, PF : 2 * PF])
                emit_upd(1, j, ys[1])
                if not last:
                    emit_q1(1, cur[1])
                a2_2 = emit_a2(2, j)
                emit_k(2, a2_2)
                emit_upd(2, j, ys[2])
                a2_3 = emit_a2(3, j)
                pend3 = (j, a2_3, ys[3])
            # epilogue: last interval's deferred pair-3 tail
            pj, pa2, py = pend3
            emit_k(3, pa2)
            emit_upd(3, pj, py)

    nc.compile()
    _CACHE[key] = nc
    return nc


def _make_in_maps(first_point, time_steps_to_predict, W1, b1, W2, b2):
    f32 = np.float32
    coef = _coef_tables(time_steps_to_predict, W1, b1, b2)
    vsc = _vsc_tables(time_steps_to_predict, W1, W2)
    W1 = np.ascontiguousarray(W1.astype(f32))
    W2 = np.ascontiguousarray(W2.astype(f32))
    w1ab = np.zeros((128, 2 * H), f32)
    w1ab[0:D, 0:H] = W1
    w1ab[D:128, H : 2 * H] = W1
    w2ab = np.zeros((H, 256), f32)
    w2ab[:, 0:D] = W2
    w2ab[:, 128 + D : 256] = W2
    # y0 transposed + padded: per batch 326 columns (last col zero)
    fpT = first_point.astype(f32).T.reshape(D, B, P)  # [D, B, P]
    y0pad = np.zeros((D, B, PF), f32)
    y0pad[:, :, 0:P] = fpT
    in_maps = []
    for c in range(NCORE):
        in_maps.append(
            {
                "y0": np.ascontiguousarray(
                    y0pad[:, c * BPC : (c + 1) * BPC, :].reshape(D, RPAD)
                ),
                "coef": np.ascontiguousarray(coef[c]),
                "vsc": np.ascontiguousarray(vsc[c]),
                "w1ab": w1ab,
                "w2ab": w2ab,
                "b1": np.ascontiguousarray(b1.astype(f32).reshape(H, 1)),
            }
        )
    return in_maps


def _assemble(core_outs):
    full = np.concatenate(core_outs, axis=2)  # [T, D, B*P]
    return np.ascontiguousarray(full.transpose(2, 1, 0)).astype(np.float32)


def run_with_results(first_point, time_steps_to_predict, W1, b1, W2, b2, trace=False):
    from concourse.bass_utils import run_bass_kernel_spmd

    first_point = np.asarray(first_point)
    time_steps_to_predict = np.asarray(time_steps_to_predict)
    W1, b1, W2, b2 = (np.asarray(a) for a in (W1, b1, W2, b2))
    fast = bool(np.all(b2 == 0))
    nc = _build_program(fast=fast)
    in_maps = _make_in_maps(first_point, time_steps_to_predict, W1, b1, W2, b2)
    res = run_bass_kernel_spmd(nc, in_maps, list(range(NCORE)), trace=trace)
    out = _assemble([res.results[c]["out"] for c in range(NCORE)])
    return out, res


def kernel(first_point, time_steps_to_predict, W1, b1, W2, b2):
    out, _ = run_with_results(first_point, time_steps_to_predict, W1, b1, W2, b2)
    return out



# revision 6
# speedup vs baseline: 1.1013x; 1.1013x over previous
"""Trainium2 Bass kernel for the DiffeqSolver problem.

Math: the reference solves dy/dt = tanh(y@W1+b1)@W2+b2 (autonomous) with
adaptive dopri5 at rtol=1e-4 for 24 per-batch time points.  The kernel
integrates with a variable-step Adams-Bashforth-2 method -- ONE f
evaluation (one tanh stage) per output interval, half the ACT-engine work
of the RK2 midpoint scheme -- bootstrapped by a single fused-midpoint RK2
step on the first interval (which also materializes k_0 = f(y_0) for
AB2's history).  Numpy reproduction of this exact scheme lands at
rel-err ~7.8e-4 vs the dopri5 reference, ~25x inside the 2e-2 gate.

AB2 recurrence with per-(batch, interval) host-precomputed coefficients
A_j = h_j(1+r_j/2), B_j = -h_j r_j/2 (r_j = h_j/h_{j-1}):
  y_{j+1} = y_j + A_j F_j + B_j F_{j-1},   F_j = k_j + b2, k_j = a1_j@W2.
With the carry s_j = y_j + B_j F_{j-1}, each interval and pair is three
vector ops spread over three engines (constraints: only DVE/ACT read
PSUM; GPSIMD/Pool has no PSUM access and no scalar_tensor_tensor, but
does have tensor_tensor add):
  u_j   = A_j*k_j + A_j*b2      [= A_j F_j]  (DVE tensor_scalar from
           PSUM for 3 pairs; ACT Identity(scale*k+bias) for pair 0 --
           ACT has slack once tanh work is halved)
  y_{j+1} = u_j + s_j                        (Pool tensor_tensor)
  s_{j+1} = (C_j/A_j)*u_j + s_j              (DVE stt, SBUF; C_j =
           A_j+B_{j+1}, so (C_j/A_j)u = C_j F exactly -- b2 rides in u
           and the generic-b2 path costs nothing anywhere)
The last interval needs no carry and collapses to y' = stt(k, A, s).

Distribution: data-parallel over the batch axis -- 8 batches per
NeuronCore in 4 pairs.  The pair state lives in one SBUF tile [128, 326]
f32r (partitions 0:64 = batch A latent dims transposed, 64:128 = batch B;
free dim padded to 326 because f32r matmuls need an even moving dim).
mm1 uses block-extended [128,128] weights so every matmul writes PSUM at
partition 0.  Each pair owns two private banks of one 8-bank PSUM tile:
q1_A in bank 2p, q1_B in bank 2p+1; k_j accumulates into bank 2p after
the tanh has read it and frees once u_j reads it.

Scheduling: every PE/ACT/DVE/Pool stream order is pinned with no-sync
(order-only) dependency edges into a software pipeline of four
pair-chains.  Steady-state ACT ladder is [a1_0, a1_1, a1_2, u_0, a1_3];
each pair's tail (k matmuls -> u -> y'/s' -> next q1) is interleaved into
the engine streams by readiness, with pairs 2/3's next-interval q1
deferred to the top of the following interval.  State writeback is one
DMA per interval and pair on the SP hwdge queue.
"""

import numpy as np
from contextlib import ExitStack

B, P, D, H, T = 64, 325, 64, 128, 24
NCORE = 8
BPC = B // NCORE  # 8 batches per core
NPAIR = BPC // 2  # 4
R = BPC * P  # 2600 rows per core
PF = P + 1  # free-dim padded to even (f32r matmul requires an even moving dim)
RPAD = BPC * PF  # per-core padded y0 width
NI = T - 1  # 23 integration intervals

# coef table column layout (see _coef_tables)
# boot block, per pair (6 cols): [h0, h0*b2, B1, B1*b2, biasA, biasB]
BOOT0 = 0
# AB block, per (j-1, pair) (3 cols): [A_j, A_j*b2, (A_j+B_{j+1})/A_j]
ABBASE = BOOT0 + 6 * NPAIR
NCOEF = ABBASE + (NI - 1) * NPAIR * 3

_CACHE = {}


def _ab_coefs(ts):
    """A[j], B[j] per (interval j, batch): variable-step AB2 coefficients.
    A[0] = h_0 (RK2 bootstrap update coefficient); B[0] unused."""
    f32 = np.float32
    dt = np.diff(ts.astype(f32), axis=0)  # [NI, B]
    A = np.empty_like(dt)
    Bc = np.zeros_like(dt)
    A[0] = dt[0]
    r = dt[1:] / dt[:-1]
    A[1:] = dt[1:] * (1 + r / 2)
    Bc[1:] = -dt[1:] * r / 2
    return A, Bc


def _coef_tables(ts, W1, b1, b2):
    """Per-core coefficient table [NCORE, 128, NCOEF] fp32."""
    f32 = np.float32
    dt = np.diff(ts.astype(f32), axis=0)  # [NI, B]
    A, Bc = _ab_coefs(ts)
    bw = (b2.astype(f32) @ W1.astype(f32)).astype(f32)  # [H]
    b1f = b1.astype(f32)
    b2f = b2.astype(f32)
    coef = np.zeros((NCORE, 128, NCOEF), f32)

    def stack(col, vA, vB):
        col[:64] = vA
        col[64:] = vB

    for c in range(NCORE):
        for p in range(NPAIR):
            bA = c * BPC + 2 * p
            bB = bA + 1
            base = BOOT0 + 6 * p
            stack(coef[c, :, base + 0], dt[0, bA], dt[0, bB])
            coef[c, :64, base + 1] = dt[0, bA] * b2f
            coef[c, 64:, base + 1] = dt[0, bB] * b2f
            stack(coef[c, :, base + 2], Bc[1, bA], Bc[1, bB])
            coef[c, :64, base + 3] = Bc[1, bA] * b2f
            coef[c, 64:, base + 3] = Bc[1, bB] * b2f
            coef[c, :, base + 4] = b1f + f32(0.5) * dt[0, bA] * bw
            coef[c, :, base + 5] = b1f + f32(0.5) * dt[0, bB] * bw
            for j in range(1, NI):
                base = ABBASE + ((j - 1) * NPAIR + p) * 3
                stack(coef[c, :, base + 0], A[j, bA], A[j, bB])
                coef[c, :64, base + 1] = A[j, bA] * b2f
                coef[c, 64:, base + 1] = A[j, bB] * b2f
                if j + 1 <= NI - 1:
                    cA = (A[j, bA] + Bc[j + 1, bA]) / A[j, bA]
                    cB = (A[j, bB] + Bc[j + 1, bB]) / A[j, bB]
                    stack(coef[c, :, base + 2], cA, cB)
    return coef


def _vsc_tables(ts, W1, W2):
    """Per-core h-scaled V = W2@W1 weight table for the bootstrap interval
    only: [NCORE, 128, NPAIR*2*H] fp32, per pair (h0A/2)*V then (h0B/2)*V."""
    f32 = np.float32
    dt = np.diff(ts.astype(f32), axis=0)  # [NI, B]
    V = (W2.astype(f32) @ W1.astype(f32)).astype(f32)  # [H, H]
    vsc = np.zeros((NCORE, 128, NPAIR * 2 * H), f32)
    for c in range(NCORE):
        for p in range(NPAIR):
            bA = c * BPC + 2 * p
            base = p * 2 * H
            vsc[c, :, base : base + H] = f32(0.5) * dt[0, bA] * V
            vsc[c, :, base + H : base + 2 * H] = f32(0.5) * dt[0, bA + 1] * V
    return vsc


def _build_program(fast=False):
    """fast=True is valid when b2 == 0 (the graded fill): the bootstrap and
    last-interval updates collapse to single stt ops."""
    key = ("nc", fast)
    if key in _CACHE:
        return _CACHE[key]

    import concourse.bacc as bacc
    import concourse.tile as tile
    import concourse.mybir as mybir

    f32 = mybir.dt.float32
    f32r = mybir.dt.float32r
    AF = mybir.ActivationFunctionType
    OP = mybir.AluOpType

    nc = bacc.Bacc(
        "TRN2",
        target_bir_lowering=False,
        debug=False,
        enable_asserts=False,
        num_devices=NCORE,
    )
    y0_d = nc.dram_tensor("y0", [D, RPAD], f32r, kind="ExternalInput").ap()
    coef_d = nc.dram_tensor("coef", [128, NCOEF], f32, kind="ExternalInput").ap()
    w1ab_d = nc.dram_tensor("w1ab", [128, 2 * H], f32r, kind="ExternalInput").ap()
    w2ab_d = nc.dram_tensor("w2ab", [H, 256], f32r, kind="ExternalInput").ap()
    b1_d = nc.dram_tensor("b1", [H, 1], f32, kind="ExternalInput").ap()
    vsc_d = nc.dram_tensor("vsc", [128, NPAIR * 2 * H], f32r, kind="ExternalInput").ap()
    out_d = nc.dram_tensor("out", [T, D, R], f32, kind="ExternalOutput").ap()

    def out_ap(j, p):
        # [2, 64, 325] view of out[j]: batch-half h, latent dim d, point q
        return out_d[j, :, 2 * p * P : (2 * p + 2) * P].rearrange(
            "d (h q) -> h d q", h=2
        )

    with tile.TileContext(nc) as tc:
        with ExitStack() as ctx:
            const = ctx.enter_context(tc.tile_pool(name="const", bufs=1))
            ypool = ctx.enter_context(tc.tile_pool(name="ypool", bufs=4))
            spool = ctx.enter_context(tc.tile_pool(name="spool", bufs=3))
            upool = ctx.enter_context(tc.tile_pool(name="upool", bufs=3))
            apool = ctx.enter_context(tc.tile_pool(name="apool", bufs=2))
            gpool = ctx.enter_context(tc.tile_pool(name="gpool", bufs=1, space="PSUM"))

            # Startup DMAs: W1 + bootstrap V table ride the (otherwise idle)
            # ACT hwdge queue; y0 pairs split across the SP and Pool queues.
            w1ab_t = const.tile([128, 2 * H], f32r, name="w1abt")
            nc.scalar.dma_start(out=w1ab_t[:], in_=w1ab_d[:])
            w1a_t = w1ab_t[:, 0:H]
            w1b_t = w1ab_t[:, H : 2 * H]
            vsc_t = const.tile([128, NPAIR * 2 * H], f32r, name="vsct")
            nc.scalar.dma_start(out=vsc_t[:], in_=vsc_d[:])

            ytiles = []
            for p in range(NPAIR):
                ytr = ypool.tile([128, PF], f32r, name=f"y{p}", tag=f"y{p}")
                deng = nc.sync if p in (0, 2) else nc.gpsimd
                deng.dma_start(
                    out=ytr[:],
                    in_=y0_d[:, 2 * p * PF : (2 * p + 2) * PF].rearrange(
                        "d (h q) -> h d q", h=2
                    ),
                )
                ytiles.append(ytr)

            b1_t = const.tile([H, 1], f32, name="b1t")
            nc.sync.dma_start(out=b1_t[:], in_=b1_d[:])
            coef_t = const.tile([128, NCOEF], f32, name="coeft")
            nc.sync.dma_start(out=coef_t[:], in_=coef_d[:])
            w2ab_t = const.tile([H, 256], f32r, name="w2abt")
            nc.sync.dma_start(out=w2ab_t[:], in_=w2ab_d[:])
            w2a_t = w2ab_t[:, 0:128]
            w2b_t = w2ab_t[:, 128:256]

            cur = list(ytiles)  # y_j tile per pair
            scur = [None] * NPAIR  # s_j carry tile per pair
            gall = gpool.tile([128, 4096], f32, name="gall", tag="gall")

            prev_on = {}

            def seq(engkey, binst):
                ins = binst.ins if hasattr(binst, "ins") else binst
                if engkey in prev_on:
                    ins.add_dependency(
                        prev_on[engkey], mybir.DependencyInfo.NO_SYNC_ONLY
                    )
                prev_on[engkey] = ins.name
                return binst

            def regions(p):
                return gall[:, 1024 * p : 1024 * p + PF], gall[
                    :, 1024 * p + 512 : 1024 * p + 512 + PF
                ]

            def emit_q1(p, y):
                gA, gB = regions(p)
                seq('PE', nc.tensor.matmul(gA, w1a_t, y[:], start=True, stop=True))
                seq('PE', nc.tensor.matmul(gB, w1b_t, y[:], start=True, stop=True))

            def emit_a1(p):
                a1 = apool.tile([128, 2 * PF], f32r, name=f"a1_{p}", tag=f"a1{p}")
                gview = gall[:, 1024 * p : 1024 * (p + 1)].rearrange(
                    "q (r c) -> q r c", r=2
                )[:, :, 0:PF]
                a1view = a1[:].rearrange("q (r c) -> q r c", r=2)
                seq('ACT', nc.scalar.activation(
                    a1view, gview, AF.Tanh, bias=b1_t[:, 0:1], scale=1.0
                ))
                return a1

            def emit_v(p, a1):
                gA, gB = regions(p)
                vA = vsc_t[:, p * 2 * H : p * 2 * H + H]
                vB = vsc_t[:, p * 2 * H + H : p * 2 * H + 2 * H]
                seq('PE', nc.tensor.matmul(
                    gA, vA, a1[:, 0:PF],
                    start=False, stop=True, skip_group_check=True,
                ))
                seq('PE', nc.tensor.matmul(
                    gB, vB, a1[:, PF : 2 * PF],
                    start=False, stop=True, skip_group_check=True,
                ))

            def emit_a2(p):
                gA, gB = regions(p)
                a2 = apool.tile([128, 2 * PF], f32r, name=f"a2_{p}", tag=f"a2{p}")
                if fast:
                    gview = gall[:, 1024 * p : 1024 * (p + 1)].rearrange(
                        "q (r c) -> q r c", r=2
                    )[:, :, 0:PF]
                    a2view = a2[:].rearrange("q (r c) -> q r c", r=2)
                    seq('ACT', nc.scalar.activation(
                        a2view, gview, AF.Tanh, bias=b1_t[:, 0:1], scale=1.0
                    ))
                else:
                    base = BOOT0 + 6 * p
                    seq('ACT', nc.scalar.activation(
                        a2[:, 0:PF], gA, AF.Tanh,
                        bias=coef_t[:, base + 4 : base + 5], scale=1.0,
                    ))
                    seq('ACT', nc.scalar.activation(
                        a2[:, PF : 2 * PF], gB, AF.Tanh,
                        bias=coef_t[:, base + 5 : base + 6], scale=1.0,
                    ))
                return a2

            def emit_kA(p, src):
                gA, gB = regions(p)
                seq('PE', nc.tensor.matmul(gA, w2a_t, src[:, 0:PF], start=True, stop=False))
                seq('PE', nc.tensor.matmul(
                    gA, w2b_t, src[:, PF : 2 * PF], start=False, stop=True
                ))

            def emit_kB(p, src):
                gA, gB = regions(p)
                seq('PE', nc.tensor.matmul(gB, w2a_t, src[:, 0:PF], start=True, stop=False))
                seq('PE', nc.tensor.matmul(
                    gB, w2b_t, src[:, PF : 2 * PF], start=False, stop=True
                ))

            def new_y(p):
                return ypool.tile([128, PF], f32r, name=f"y{p}", tag=f"y{p}")

            def new_s(p):
                return spool.tile([128, PF], f32, name=f"s{p}", tag=f"s{p}")

            def emit_stt_psum(p, kv, c_ap, c2_ap, base_t, dst):
                # dst = c*k + base (+ c*b2 via ts+tt when not fast)
                if fast:
                    seq('DVE', nc.vector.scalar_tensor_tensor(
                        dst[:], kv, c_ap, base_t, OP.mult, OP.add
                    ))
                else:
                    tmp = upool.tile([128, PF], f32, name=f"t{p}", tag=f"t{p}")
                    seq('DVE', nc.vector.tensor_scalar(
                        tmp[:], kv, c_ap, c2_ap, OP.mult, OP.add
                    ))
                    seq('DVE', nc.vector.tensor_tensor(
                        dst[:], tmp[:], base_t, OP.add
                    ))

            # ---- prologue: q1 of the bootstrap interval ----
            for p in range(NPAIR):
                emit_q1(p, ytiles[p])

            # ---- bootstrap interval (j=0): fused-midpoint RK2 + k_0 ----
            a1s = [emit_a1(p) for p in range(NPAIR)]
            for p in range(NPAIR):
                emit_v(p, a1s[p])
            a2s = [emit_a2(p) for p in range(NPAIR)]
            for p in range(NPAIR):
                emit_kA(p, a2s[p])  # k_mid -> bank A
                emit_kB(p, a1s[p])  # k_0   -> bank B
            for p in range(NPAIR):
                gA, gB = regions(p)
                base = BOOT0 + 6 * p
                # y_1 = y_0 + h0*(k_mid + b2)
                y1 = new_y(p)
                emit_stt_psum(p, gA, coef_t[:, base : base + 1],
                              coef_t[:, base + 1 : base + 2],
                              ytiles[p][:].bitcast(f32), y1)
                cur[p] = y1
                # s_1 = y_1 + B1*(k_0 + b2)
                s1 = new_s(p)
                emit_stt_psum(p, gB, coef_t[:, base + 2 : base + 3],
                              coef_t[:, base + 3 : base + 4],
                              y1[:].bitcast(f32), s1)
                scur[p] = s1
                nc.sync.dma_start(out=out_ap(1, p), in_=y1[:, 0:P].bitcast(f32))
                emit_q1(p, y1)

            # ---- AB2 intervals j=1..22 ----
            pend = []  # deferred next-interval q1 emissions [(p, ytile)]
            for j in range(1, NI):
                last = j == NI - 1
                if j == 2:
                    # t=0 outputs, deferred past startup
                    for p in range(NPAIR):
                        nc.sync.dma_start(
                            out=out_ap(0, p), in_=ytiles[p][:, 0:P].bitcast(f32)
                        )
                lo = list(pend)
                pend = []

                def coefs(p):
                    base = ABBASE + ((j - 1) * NPAIR + p) * 3
                    return (coef_t[:, base : base + 1],
                            coef_t[:, base + 1 : base + 2],
                            coef_t[:, base + 2 : base + 3])

                def tail(p, nxt_inline):
                    gA, gB = regions(p)
                    aap, abap, cap = coefs(p)
                    s = scur[p]
                    ynew = new_y(p)
                    if last:
                        # y' = s + A*(k + b2): single stt (fast) / ts+tt
                        emit_stt_psum(p, gA, aap, abap, s[:], ynew)
                        cur[p] = ynew
                    else:
                        # u = A*k + A*b2  (pair 0 on ACT, others on DVE)
                        u = upool.tile([128, PF], f32, name=f"u{p}", tag=f"u{p}")
                        if p == 0:
                            seq('ACT', nc.scalar.activation(
                                u[:], gA, AF.Identity,
                                bias=abap, scale=aap,
                            ))
                        else:
                            seq('DVE', nc.vector.tensor_scalar(
                                u[:], gA, aap, abap, OP.mult, OP.add
                            ))
                        # y' = u + s  (Pool)
                        seq('POOL', nc.gpsimd.tensor_tensor(
                            ynew[:], u[:], s[:], OP.add
                        ))
                        cur[p] = ynew
                        # s' = (C/A)*u + s  (DVE, SBUF)
                        snew = new_s(p)
                        seq('DVE', nc.vector.scalar_tensor_tensor(
                            snew[:], u[:], cap, s[:], OP.mult, OP.add
                        ))
                        scur[p] = snew
                    nc.sync.dma_start(
                        out=out_ap(j + 1, p), in_=ynew[:, 0:P].bitcast(f32)
                    )
                    if not last:
                        if nxt_inline:
                            emit_q1(p, ynew)
                        else:
                            pend.append((p, ynew))

                if lo:
                    emit_q1(*lo[0])
                a1_0 = emit_a1(0)
                emit_kA(0, a1_0)
                if len(lo) > 1:
                    emit_q1(*lo[1])
                a1_1 = emit_a1(1)
                emit_kA(1, a1_1)
                a1_2 = emit_a1(2)
                emit_kA(2, a1_2)
                tail(0, True)
                a1_3 = emit_a1(3)
                emit_kA(3, a1_3)
                tail(1, True)
                tail(2, False)
                tail(3, False)

    nc.compile()
    _CACHE[key] = nc
    return nc


def _make_in_maps(first_point, time_steps_to_predict, W1, b1, W2, b2):
    f32 = np.float32
    coef = _coef_tables(time_steps_to_predict, W1, b1, b2)
    vsc = _vsc_tables(time_steps_to_predict, W1, W2)
    W1 = np.ascontiguousarray(W1.astype(f32))
    W2 = np.ascontiguousarray(W2.astype(f32))
    w1ab = np.zeros((128, 2 * H), f32)
    w1ab[0:D, 0:H] = W1
    w1ab[D:128, H : 2 * H] = W1
    w2ab = np.zeros((H, 256), f32)
    w2ab[:, 0:D] = W2
    w2ab[:, 128 + D : 256] = W2
    # y0 transposed + padded: per batch 326 columns (last col zero)
    fpT = first_point.astype(f32).T.reshape(D, B, P)  # [D, B, P]
    y0pad = np.zeros((D, B, PF), f32)
    y0pad[:, :, 0:P] = fpT
    in_maps = []
    for c in range(NCORE):
        in_maps.append(
            {
                "y0": np.ascontiguousarray(
                    y0pad[:, c * BPC : (c + 1) * BPC, :].reshape(D, RPAD)
                ),
                "coef": np.ascontiguousarray(coef[c]),
                "vsc": np.ascontiguousarray(vsc[c]),
                "w1ab": w1ab,
                "w2ab": w2ab,
                "b1": np.ascontiguousarray(b1.astype(f32).reshape(H, 1)),
            }
        )
    return in_maps


def _assemble(core_outs):
    full = np.concatenate(core_outs, axis=2)  # [T, D, B*P]
    return np.ascontiguousarray(full.transpose(2, 1, 0)).astype(np.float32)


def run_with_results(first_point, time_steps_to_predict, W1, b1, W2, b2, trace=False):
    from concourse.bass_utils import run_bass_kernel_spmd

    first_point = np.asarray(first_point)
    time_steps_to_predict = np.asarray(time_steps_to_predict)
    W1, b1, W2, b2 = (np.asarray(a) for a in (W1, b1, W2, b2))
    fast = bool(np.all(b2 == 0))
    nc = _build_program(fast=fast)
    in_maps = _make_in_maps(first_point, time_steps_to_predict, W1, b1, W2, b2)
    res = run_bass_kernel_spmd(nc, in_maps, list(range(NCORE)), trace=trace)
    out = _assemble([res.results[c]["out"] for c in range(NCORE)])
    return out, res


def kernel(first_point, time_steps_to_predict, W1, b1, W2, b2):
    out, _ = run_with_results(first_point, time_steps_to_predict, W1, b1, W2, b2)
    return out


# revision 12
# speedup vs baseline: 1.1553x; 1.0490x over previous
"""Trainium2 Bass kernel for the DiffeqSolver problem.

Math: the reference solves dy/dt = tanh(y@W1+b1)@W2+b2 (autonomous) with
adaptive dopri5 at rtol=1e-4 for 24 per-batch time points.  The kernel
integrates with a variable-step Adams-Bashforth-2 method -- ONE f
evaluation (one tanh stage) per output interval, half the ACT-engine work
of the RK2 midpoint scheme -- bootstrapped by a single fused-midpoint RK2
step on the first interval (which also materializes k_0 = f(y_0) for
AB2's history).  Numpy reproduction of this exact scheme lands at
rel-err ~7.8e-4 vs the dopri5 reference, ~25x inside the 2e-2 gate.

AB2 recurrence with per-(batch, interval) host-precomputed coefficients
A_j = h_j(1+r_j/2), B_j = -h_j r_j/2 (r_j = h_j/h_{j-1}):
  y_{j+1} = y_j + A_j F_j + B_j F_{j-1},   F_j = k_j + b2, k_j = a1_j@W2.
With the carry s_j = y_j + B_j F_{j-1}, each interval and pair is two
fused scalar_tensor_tensor ops reading k_j straight from its PSUM bank:
  y_{j+1} = A_j*k_j + s_j          (DVE)
  s_{j+1} = C_j*k_j + s_j          (C_j = A_j + B_{j+1})
Both land on DVE (only DVE/ACT can read PSUM, and GPSIMD/Pool has no
scalar_tensor_tensor), which would leave DVE ~25% busier than ACT; to
rebalance, pair 0's carry is built from an ACT Identity op
(u = C_0*k + C_0*b2, per-partition scale/bias, slotted mid-ladder) plus
a Pool tensor_tensor add -- both off the loop-carried critical path.
This keeps every pair's critical chain the short one:
  tanh -> k matmuls -> y' stt -> next q1   (~2.8 us)
The last interval needs no carry and collapses to the y' op alone.
b2 enters only through u/ts second scalars, so the generic-b2 path
costs one extra DVE op per update and the graded b2=0 path none.

Distribution: data-parallel over the batch axis -- 8 batches per
NeuronCore in 4 pairs.  The pair state lives in one SBUF tile [128, 326]
f32r (partitions 0:64 = batch A latent dims transposed, 64:128 = batch B;
free dim padded to 326 because f32r matmuls need an even moving dim).
mm1 uses block-extended [128,128] weights so every matmul writes PSUM at
partition 0.  Each pair owns two private banks of one 8-bank PSUM tile:
q1_A in bank 2p, q1_B in bank 2p+1; k_j accumulates into bank 2p after
the tanh has read it and frees once u_j reads it.

Scheduling: every PE/ACT/DVE/Pool stream order is pinned with no-sync
(order-only) dependency edges into a software pipeline of four
pair-chains.  Steady-state ACT ladder is [a1_0, a1_1, u_0, a1_2, a1_3];
the PE stream runs pairs 2/3's deferred q1 first (their states complete
early now, so nothing head-of-line blocks), then the k matmuls and
pairs 0/1's inline q1 by readiness.  State writeback is one DMA per
interval and pair on the SP hwdge queue.
"""

import numpy as np
from contextlib import ExitStack

B, P, D, H, T = 64, 325, 64, 128, 24
NCORE = 8
BPC = B // NCORE  # 8 batches per core
NPAIR = BPC // 2  # 4
R = BPC * P  # 2600 rows per core
PF = P + 1  # free-dim padded to even (f32r matmul requires an even moving dim)
RPAD = BPC * PF  # per-core padded y0 width
NI = T - 1  # 23 integration intervals

# coef table column layout (see _coef_tables)
# boot block, per pair (6 cols): [h0, h0*b2, B1, B1*b2, biasA, biasB]
BOOT0 = 0
# AB block, per (j-1, pair) (4 cols): [A_j, A_j*b2, C_j, C_j*b2]
# where C_j = A_j + B_{j+1} (the carry-step coefficient on F_j)
ABBASE = BOOT0 + 6 * NPAIR
NCOEF = ABBASE + (NI - 1) * NPAIR * 4

_CACHE = {}


def _ab_coefs(ts):
    """A[j], B[j] per (interval j, batch): variable-step AB2 coefficients.
    A[0] = h_0 (RK2 bootstrap update coefficient); B[0] unused."""
    f32 = np.float32
    dt = np.diff(ts.astype(f32), axis=0)  # [NI, B]
    A = np.empty_like(dt)
    Bc = np.zeros_like(dt)
    A[0] = dt[0]
    r = dt[1:] / dt[:-1]
    A[1:] = dt[1:] * (1 + r / 2)
    Bc[1:] = -dt[1:] * r / 2
    return A, Bc


def _coef_tables(ts, W1, b1, b2):
    """Per-core coefficient table [NCORE, 128, NCOEF] fp32."""
    f32 = np.float32
    dt = np.diff(ts.astype(f32), axis=0)  # [NI, B]
    A, Bc = _ab_coefs(ts)
    bw = (b2.astype(f32) @ W1.astype(f32)).astype(f32)  # [H]
    b1f = b1.astype(f32)
    b2f = b2.astype(f32)
    coef = np.zeros((NCORE, 128, NCOEF), f32)

    def stack(col, vA, vB):
        col[:64] = vA
        col[64:] = vB

    for c in range(NCORE):
        for p in range(NPAIR):
            bA = c * BPC + 2 * p
            bB = bA + 1
            base = BOOT0 + 6 * p
            stack(coef[c, :, base + 0], dt[0, bA], dt[0, bB])
            coef[c, :64, base + 1] = dt[0, bA] * b2f
            coef[c, 64:, base + 1] = dt[0, bB] * b2f
            stack(coef[c, :, base + 2], Bc[1, bA], Bc[1, bB])
            coef[c, :64, base + 3] = Bc[1, bA] * b2f
            coef[c, 64:, base + 3] = Bc[1, bB] * b2f
            coef[c, :, base + 4] = b1f + f32(0.5) * dt[0, bA] * bw
            coef[c, :, base + 5] = b1f + f32(0.5) * dt[0, bB] * bw
            for j in range(1, NI):
                base = ABBASE + ((j - 1) * NPAIR + p) * 4
                stack(coef[c, :, base + 0], A[j, bA], A[j, bB])
                coef[c, :64, base + 1] = A[j, bA] * b2f
                coef[c, 64:, base + 1] = A[j, bB] * b2f
                if j + 1 <= NI - 1:
                    cA = A[j, bA] + Bc[j + 1, bA]
                    cB = A[j, bB] + Bc[j + 1, bB]
                    stack(coef[c, :, base + 2], cA, cB)
                    coef[c, :64, base + 3] = cA * b2f
                    coef[c, 64:, base + 3] = cB * b2f
    return coef


def _vsc_tables(ts, W1, W2):
    """Per-core h-scaled V = W2@W1 weight table for the bootstrap interval
    only: [NCORE, 128, NPAIR*2*H] fp32, per pair (h0A/2)*V then (h0B/2)*V."""
    f32 = np.float32
    dt = np.diff(ts.astype(f32), axis=0)  # [NI, B]
    V = (W2.astype(f32) @ W1.astype(f32)).astype(f32)  # [H, H]
    vsc = np.zeros((NCORE, 128, NPAIR * 2 * H), f32)
    for c in range(NCORE):
        for p in range(NPAIR):
            bA = c * BPC + 2 * p
            base = p * 2 * H
            vsc[c, :, base : base + H] = f32(0.5) * dt[0, bA] * V
            vsc[c, :, base + H : base + 2 * H] = f32(0.5) * dt[0, bA + 1] * V
    return vsc


def _build_program(fast=False):
    """fast=True is valid when b2 == 0 (the graded fill): the bootstrap and
    last-interval updates collapse to single stt ops."""
    key = ("nc", fast)
    if key in _CACHE:
        return _CACHE[key]

    import concourse.bacc as bacc
    import concourse.tile as tile
    import concourse.mybir as mybir

    f32 = mybir.dt.float32
    f32r = mybir.dt.float32r
    AF = mybir.ActivationFunctionType
    OP = mybir.AluOpType

    nc = bacc.Bacc(
        "TRN2",
        target_bir_lowering=False,
        debug=False,
        enable_asserts=False,
        num_devices=NCORE,
    )
    y0_d = nc.dram_tensor("y0", [D, RPAD], f32r, kind="ExternalInput").ap()
    coef_d = nc.dram_tensor("coef", [128, NCOEF], f32, kind="ExternalInput").ap()
    w1ab_d = nc.dram_tensor("w1ab", [128, 2 * H], f32r, kind="ExternalInput").ap()
    w2ab_d = nc.dram_tensor("w2ab", [H, 256], f32r, kind="ExternalInput").ap()
    b1_d = nc.dram_tensor("b1", [H, 1], f32, kind="ExternalInput").ap()
    vsc_d = nc.dram_tensor("vsc", [128, NPAIR * 2 * H], f32r, kind="ExternalInput").ap()
    out_d = nc.dram_tensor("out", [T, D, R], f32, kind="ExternalOutput").ap()

    def out_ap(j, p):
        # [2, 64, 325] view of out[j]: batch-half h, latent dim d, point q
        return out_d[j, :, 2 * p * P : (2 * p + 2) * P].rearrange(
            "d (h q) -> h d q", h=2
        )

    with tile.TileContext(nc) as tc:
        with ExitStack() as ctx:
            const = ctx.enter_context(tc.tile_pool(name="const", bufs=1))
            ypool = ctx.enter_context(tc.tile_pool(name="ypool", bufs=4))
            spool = ctx.enter_context(tc.tile_pool(name="spool", bufs=3))
            upool = ctx.enter_context(tc.tile_pool(name="upool", bufs=3))
            apool = ctx.enter_context(tc.tile_pool(name="apool", bufs=2))
            gpool = ctx.enter_context(tc.tile_pool(name="gpool", bufs=1, space="PSUM"))

            # Startup DMAs: W1 + bootstrap V table ride the (otherwise idle)
            # ACT hwdge queue; y0 pairs split across the SP and Pool queues.
            w1ab_t = const.tile([128, 2 * H], f32r, name="w1abt")
            nc.scalar.dma_start(out=w1ab_t[:], in_=w1ab_d[:])
            w1a_t = w1ab_t[:, 0:H]
            w1b_t = w1ab_t[:, H : 2 * H]
            vsc_t = const.tile([128, NPAIR * 2 * H], f32r, name="vsct")
            nc.scalar.dma_start(out=vsc_t[:], in_=vsc_d[:])

            ytiles = []
            for p in range(NPAIR):
                ytr = ypool.tile([128, PF], f32r, name=f"y{p}", tag=f"y{p}")
                deng = nc.sync if p in (0, 2) else nc.gpsimd
                deng.dma_start(
                    out=ytr[:],
                    in_=y0_d[:, 2 * p * PF : (2 * p + 2) * PF].rearrange(
                        "d (h q) -> h d q", h=2
                    ),
                )
                ytiles.append(ytr)

            b1_t = const.tile([H, 1], f32, name="b1t")
            nc.sync.dma_start(out=b1_t[:], in_=b1_d[:])
            coef_t = const.tile([128, NCOEF], f32, name="coeft")
            nc.sync.dma_start(out=coef_t[:], in_=coef_d[:])
            w2ab_t = const.tile([H, 256], f32r, name="w2abt")
            nc.sync.dma_start(out=w2ab_t[:], in_=w2ab_d[:])
            w2a_t = w2ab_t[:, 0:128]
            w2b_t = w2ab_t[:, 128:256]

            cur = list(ytiles)  # y_j tile per pair
            scur = [None] * NPAIR  # s_j carry tile per pair
            gall = gpool.tile([128, 4096], f32, name="gall", tag="gall")

            prev_on = {}

            def seq(engkey, binst):
                ins = binst.ins if hasattr(binst, "ins") else binst
                if engkey in prev_on:
                    ins.add_dependency(
                        prev_on[engkey], mybir.DependencyInfo.NO_SYNC_ONLY
                    )
                prev_on[engkey] = ins.name
                return binst

            def regions(p):
                return gall[:, 1024 * p : 1024 * p + PF], gall[
                    :, 1024 * p + 512 : 1024 * p + 512 + PF
                ]

            def emit_q1(p, y):
                gA, gB = regions(p)
                seq('PE', nc.tensor.matmul(gA, w1a_t, y[:], start=True, stop=True))
                seq('PE', nc.tensor.matmul(gB, w1b_t, y[:], start=True, stop=True))

            def emit_a1(p):
                a1 = apool.tile([128, 2 * PF], f32r, name=f"a1_{p}", tag=f"a1{p}")
                gview = gall[:, 1024 * p : 1024 * (p + 1)].rearrange(
                    "q (r c) -> q r c", r=2
                )[:, :, 0:PF]
                a1view = a1[:].rearrange("q (r c) -> q r c", r=2)
                seq('ACT', nc.scalar.activation(
                    a1view, gview, AF.Tanh, bias=b1_t[:, 0:1], scale=1.0
                ))
                return a1

            def emit_v(p, a1):
                gA, gB = regions(p)
                vA = vsc_t[:, p * 2 * H : p * 2 * H + H]
                vB = vsc_t[:, p * 2 * H + H : p * 2 * H + 2 * H]
                seq('PE', nc.tensor.matmul(
                    gA, vA, a1[:, 0:PF],
                    start=False, stop=True, skip_group_check=True,
                ))
                seq('PE', nc.tensor.matmul(
                    gB, vB, a1[:, PF : 2 * PF],
                    start=False, stop=True, skip_group_check=True,
                ))

            def emit_a2(p):
                gA, gB = regions(p)
                a2 = apool.tile([128, 2 * PF], f32r, name=f"a2_{p}", tag=f"a2{p}")
                if fast:
                    gview = gall[:, 1024 * p : 1024 * (p + 1)].rearrange(
                        "q (r c) -> q r c", r=2
                    )[:, :, 0:PF]
                    a2view = a2[:].rearrange("q (r c) -> q r c", r=2)
                    seq('ACT', nc.scalar.activation(
                        a2view, gview, AF.Tanh, bias=b1_t[:, 0:1], scale=1.0
                    ))
                else:
                    base = BOOT0 + 6 * p
                    seq('ACT', nc.scalar.activation(
                        a2[:, 0:PF], gA, AF.Tanh,
                        bias=coef_t[:, base + 4 : base + 5], scale=1.0,
                    ))
                    seq('ACT', nc.scalar.activation(
                        a2[:, PF : 2 * PF], gB, AF.Tanh,
                        bias=coef_t[:, base + 5 : base + 6], scale=1.0,
                    ))
                return a2

            def emit_kA(p, src):
                gA, gB = regions(p)
                seq('PE', nc.tensor.matmul(gA, w2a_t, src[:, 0:PF], start=True, stop=False))
                seq('PE', nc.tensor.matmul(
                    gA, w2b_t, src[:, PF : 2 * PF], start=False, stop=True
                ))

            def emit_kB(p, src):
                gA, gB = regions(p)
                seq('PE', nc.tensor.matmul(gB, w2a_t, src[:, 0:PF], start=True, stop=False))
                seq('PE', nc.tensor.matmul(
                    gB, w2b_t, src[:, PF : 2 * PF], start=False, stop=True
                ))

            def new_y(p):
                return ypool.tile([128, PF], f32r, name=f"y{p}", tag=f"y{p}")

            def new_s(p):
                return spool.tile([128, PF], f32, name=f"s{p}", tag=f"s{p}")

            def emit_stt_psum(p, kv, c_ap, c2_ap, base_t, dst):
                # dst = c*k + base (+ c*b2 via ts+tt when not fast)
                if fast:
                    seq('DVE', nc.vector.scalar_tensor_tensor(
                        dst[:], kv, c_ap, base_t, OP.mult, OP.add
                    ))
                else:
                    tmp = upool.tile([128, PF], f32, name=f"t{p}", tag=f"t{p}")
                    seq('DVE', nc.vector.tensor_scalar(
                        tmp[:], kv, c_ap, c2_ap, OP.mult, OP.add
                    ))
                    seq('DVE', nc.vector.tensor_tensor(
                        dst[:], tmp[:], base_t, OP.add
                    ))

            # ---- prologue: q1 of the bootstrap interval ----
            for p in range(NPAIR):
                emit_q1(p, ytiles[p])

            # ---- bootstrap interval (j=0): fused-midpoint RK2 + k_0 ----
            a1s = [emit_a1(p) for p in range(NPAIR)]
            for p in range(NPAIR):
                emit_v(p, a1s[p])
            a2s = [emit_a2(p) for p in range(NPAIR)]
            for p in range(NPAIR):
                emit_kA(p, a2s[p])  # k_mid -> bank A
                emit_kB(p, a1s[p])  # k_0   -> bank B
            for p in range(NPAIR):
                gA, gB = regions(p)
                base = BOOT0 + 6 * p
                # y_1 = y_0 + h0*(k_mid + b2)
                y1 = new_y(p)
                emit_stt_psum(p, gA, coef_t[:, base : base + 1],
                              coef_t[:, base + 1 : base + 2],
                              ytiles[p][:].bitcast(f32), y1)
                cur[p] = y1
                # s_1 = y_1 + B1*(k_0 + b2)
                s1 = new_s(p)
                emit_stt_psum(p, gB, coef_t[:, base + 2 : base + 3],
                              coef_t[:, base + 3 : base + 4],
                              y1[:].bitcast(f32), s1)
                scur[p] = s1
                nc.sync.dma_start(out=out_ap(1, p), in_=y1[:, 0:P].bitcast(f32))
                emit_q1(p, y1)

            # ---- AB2 intervals j=1..22 ----
            pend = []  # deferred next-interval q1 emissions [(p, ytile)]
            for j in range(1, NI):
                last = j == NI - 1
                if j == 2:
                    # t=0 outputs, deferred past startup
                    for p in range(NPAIR):
                        nc.sync.dma_start(
                            out=out_ap(0, p), in_=ytiles[p][:, 0:P].bitcast(f32)
                        )
                lo = list(pend)
                pend = []

                def coefs(p):
                    base = ABBASE + ((j - 1) * NPAIR + p) * 4
                    return tuple(
                        coef_t[:, base + i : base + i + 1] for i in range(4)
                    )

                def tail(p, nxt_inline):
                    gA, gB = regions(p)
                    aap, abap, cap, cbap = coefs(p)
                    s = scur[p]
                    ynew = new_y(p)
                    # y' = s + A*(k + b2): single stt (fast) / ts+tt
                    emit_stt_psum(p, gA, aap, abap, s[:], ynew)
                    cur[p] = ynew
                    if not last:
                        # s' = s + C*(k + b2)
                        snew = new_s(p)
                        if p == 0:
                            # rebalance: ACT builds u = C*k + C*b2 from PSUM
                            # (Identity, per-partition scale+bias), Pool adds
                            # the carry -- both off the critical chain
                            u = upool.tile([128, PF], f32, name="u0", tag="u0")
                            seq('ACT', nc.scalar.activation(
                                u[:], gA, AF.Identity, bias=cbap, scale=cap,
                            ))
                            seq('POOL', nc.gpsimd.tensor_tensor(
                                snew[:], u[:], s[:], OP.add
                            ))
                        else:
                            emit_stt_psum(p, gA, cap, cbap, s[:], snew)
                        scur[p] = snew
                    nc.sync.dma_start(
                        out=out_ap(j + 1, p), in_=ynew[:, 0:P].bitcast(f32)
                    )
                    if not last:
                        if nxt_inline:
                            emit_q1(p, ynew)
                        else:
                            pend.append((p, ynew))

                # leftover q1s first: their states completed early in the
                # previous interval, so they are ready at interval top
                for item in lo:
                    emit_q1(*item)
                a1_0 = emit_a1(0)
                emit_kA(0, a1_0)
                a1_1 = emit_a1(1)
                emit_kA(1, a1_1)
                a1_2 = emit_a1(2)
                emit_kA(2, a1_2)
                tail(0, True)
                a1_3 = emit_a1(3)
                emit_kA(3, a1_3)
                tail(1, True)
                tail(2, False)
                tail(3, False)

    nc.compile()
    _CACHE[key] = nc
    return nc


def _make_in_maps(first_point, time_steps_to_predict, W1, b1, W2, b2):
    f32 = np.float32
    coef = _coef_tables(time_steps_to_predict, W1, b1, b2)
    vsc = _vsc_tables(time_steps_to_predict, W1, W2)
    W1 = np.ascontiguousarray(W1.astype(f32))
    W2 = np.ascontiguousarray(W2.astype(f32))
    w1ab = np.zeros((128, 2 * H), f32)
    w1ab[0:D, 0:H] = W1
    w1ab[D:128, H : 2 * H] = W1
    w2ab = np.zeros((H, 256), f32)
    w2ab[:, 0:D] = W2
    w2ab[:, 128 + D : 256] = W2
    # y0 transposed + padded: per batch 326 columns (last col zero)
    fpT = first_point.astype(f32).T.reshape(D, B, P)  # [D, B, P]
    y0pad = np.zeros((D, B, PF), f32)
    y0pad[:, :, 0:P] = fpT
    in_maps = []
    for c in range(NCORE):
        in_maps.append(
            {
                "y0": np.ascontiguousarray(
                    y0pad[:, c * BPC : (c + 1) * BPC, :].reshape(D, RPAD)
                ),
                "coef": np.ascontiguousarray(coef[c]),
                "vsc": np.ascontiguousarray(vsc[c]),
                "w1ab": w1ab,
                "w2ab": w2ab,
                "b1": np.ascontiguousarray(b1.astype(f32).reshape(H, 1)),
            }
        )
    return in_maps


def _assemble(core_outs):
    full = np.concatenate(core_outs, axis=2)  # [T, D, B*P]
    return np.ascontiguousarray(full.transpose(2, 1, 0)).astype(np.float32)


def run_with_results(first_point, time_steps_to_predict, W1, b1, W2, b2, trace=False):
    from concourse.bass_utils import run_bass_kernel_spmd

    first_point = np.asarray(first_point)
    time_steps_to_predict = np.asarray(time_steps_to_predict)
    W1, b1, W2, b2 = (np.asarray(a) for a in (W1, b1, W2, b2))
    fast = bool(np.all(b2 == 0))
    nc = _build_program(fast=fast)
    in_maps = _make_in_maps(first_point, time_steps_to_predict, W1, b1, W2, b2)
    res = run_bass_kernel_spmd(nc, in_maps, list(range(NCORE)), trace=trace)
    out = _assemble([res.results[c]["out"] for c in range(NCORE)])
    return out, res


def kernel(first_point, time_steps_to_predict, W1, b1, W2, b2):
    out, _ = run_with_results(first_point, time_steps_to_predict, W1, b1, W2, b2)
    return out
